# revision 1
# baseline (speedup 1.0000x reference)
"""Causal multi-head attention on 8 trn2 NeuronCores.

Problem: B=4, S=2048, D=1024, H=16 heads (HD=64), causal softmax attention
with out-projection + bias.

Sharding (tensor-parallel over heads, data-parallel over batch):
  core c -> batch b = c // 2, head half = c % 2 (8 of 16 heads, 512 dims).
  Every core runs the IDENTICAL program on different data:
    - xt   : x[b].T                  [1024, 2048] (host pre-transposed)
    - wq/wk/wv : W[:, half slice]    [1024, 512]
    - wot  : Wo[:, half slice].T     [512, 1024]
    - bot  : bias, transposed layout [128, 8] (bot[p, c] = bo[c*128+p]);
             real on even cores, zeros on odd cores.
  Core output: partial TRANSPOSED out-projection [1024, 2048]; host sums the
  two partials per batch and transposes (row-parallel out_proj reduction).

Kernel (per core), flash-style with transposed scores, all matmul operands
bf16 (PSUM accumulation stays fp32; rel-err gate is 2e-2):
  QT = wq.T @ x.T   [512, 2048]; KT likewise; V = x @ wv [2048, 512]
  augmented with a ones column per head (the 65th row of the ctx matmul
  then yields the softmax denominator Z).
  scoresT[k, q] per head = KT_h-slice^T @ QT_h -> psum [128 keys, q].
  Head PAIRS are computed concurrently via PE row tiling (64x128 mode,
  tile_position (0,0)/(64,0)) since the contraction dim is HD=64.
  exp on ACT (scale = 1/8 folded in) -> bf16 ex tiles; causal masking is
  applied POST-exp only on the 128-wide diagonal blocks via the TENSOR_MASK
  custom DVE op (zero where q < k), so exp never waits on mask work.
  ctx^T accumulated over key tiles (lhsT = V|ones, M=65), normalized by
  1/Z (DVE reciprocal + gpsimd partition_broadcast), then TRANSPOSED
  out-projection outT[o, q] = wot.T-chunk @ ctxT with the bias added as a
  per-partition ACT bias during the PSUM->SBUF eviction.
"""

import os
from contextlib import ExitStack

import numpy as np

import concourse.mybir as mybir
import concourse.tile as tile
from concourse import bacc
from concourse.bass_utils import run_bass_kernel_spmd

B, S, D, H = 4, 2048, 1024, 16
HD = 64          # head dim
DL = 512         # local head dims per core (8 heads)
HH = 8           # local heads
P = 128
QC = 512         # q chunk (moving free dim)
N_QC = S // QC   # 4
N_DI = D // P    # 8
N_DL = DL // P   # 4
N_ST = S // P    # 16 seq tiles
VW = HD + 1      # 65: V columns + ones column
O_ = 1024        # output dims (full)

F32 = mybir.dt.float32
_MM_DT = {
    "f32": mybir.dt.float32,
    "f32r": mybir.dt.float32r,
    "bf16": mybir.dt.bfloat16,
}[os.environ.get("MHA_MM_DT", "bf16")]
MMT = _MM_DT  # dtype for every tile that feeds the PE


def _ldw(nc, dst, src_ap):
    """DMA-load a matmul-operand tile, casting f32 -> MMT when needed.

    Only gpsimd (SWDGE) DMAs may cast; round-robin over the SWDGE queues so
    the casts run in parallel.
    """
    if MMT == F32:
        nc.sync.dma_start(dst, src_ap)
    else:
        nc.gpsimd.dma_start(dst, src_ap)


def build_nc():
    nc = bacc.Bacc("TRN2", target_bir_lowering=False, debug=False,
                   num_devices=N_CORES, num_swdge_queues=4)
    xt = nc.dram_tensor("xt", [D, S], F32, kind="ExternalInput").ap()
    wq = nc.dram_tensor("wq", [D, DL], F32, kind="ExternalInput").ap()
    wk = nc.dram_tensor("wk", [D, DL], F32, kind="ExternalInput").ap()
    wv = nc.dram_tensor("wv", [D, DL], F32, kind="ExternalInput").ap()
    wot = nc.dram_tensor("wot", [DL, O_], F32, kind="ExternalInput").ap()
    bot = nc.dram_tensor("bot", [P, O_ // P], F32, kind="ExternalInput").ap()
    out = nc.dram_tensor("out", [O_, S], F32, kind="ExternalOutput").ap()

    repeat = int(os.environ.get("MHA_REPEAT", "1"))
    hwloop = int(os.environ.get("MHA_HWLOOP", "0"))
    with tile.TileContext(nc) as tc:
        if hwloop > 1:
            with tc.For_i(0, hwloop, 1):
                _emit(nc, tc, xt, wq, wk, wv, wot, bot, out)
        else:
            for _ in range(repeat):
                _emit(nc, tc, xt, wq, wk, wv, wot, bot, out)
    nc.compile()
    return nc


N_CORES = 8


def _emit(nc, tc, xt, wq, wk, wv, wot, bot, out):
    Exp = mybir.ActivationFunctionType.Exp
    Ident = mybir.ActivationFunctionType.Identity
    mult = mybir.AluOpType.mult

    NOMASK = bool(os.environ.get("MHA_NOMASK"))
    NOCTX = bool(os.environ.get("MHA_NOCTX"))
    # Defaults are the hardware-validated configuration: pre-exp psum mask
    # adds and DVE bias (the tensor_mask / Identity-bias ISA paths crashed
    # the device); row-tiled score pairs unless MHA_NOTILE.
    NOTILE = bool(os.environ.get("MHA_NOTILE"))
    OLDMASK = not os.environ.get("MHA_NEWMASK")
    DVEBIAS = not os.environ.get("MHA_ACTBIAS")
    STAGGER = int(os.environ.get("MHA_STAGGER", "4"))
    TRICKLE = int(os.environ.get("MHA_TRICKLE", "2"))

    with ExitStack() as ctx:
        # ---- small constants -----------------------------------------------
        consts = ctx.enter_context(tc.tile_pool(name="consts", bufs=1))
        ones_f = consts.tile([P, HH], F32, tag="ones_f")
        nc.gpsimd.memset(ones_f[:], 1.0)
        bot_sb = consts.tile([P, O_ // P], F32, tag="bot_sb")
        nc.sync.dma_start(bot_sb[:], bot[:])
        # post-exp causal zeroing helpers: keep ex[k, q] where -q - 1 < -k.
        mlen_c = moffs = None
        if not OLDMASK:
            mlen_c = consts.tile([P, 1], F32, tag="mlen_c")
            nc.gpsimd.iota(mlen_c[:], pattern=[[0, 1]], base=0,
                           channel_multiplier=-1,
                           allow_small_or_imprecise_dtypes=True)
            moffs = consts.tile([P, P], MMT, tag="moffs")
            nc.gpsimd.iota(moffs[:], pattern=[[-1, P]], base=0,
                           channel_multiplier=0,
                           allow_small_or_imprecise_dtypes=True)
        mask128 = consts.tile([P, P], F32, tag="mask128")
        nc.gpsimd.memset(mask128[:], 0.0)
        nc.gpsimd.affine_select(
            out=mask128[:], in_=mask128[:],
            pattern=[[1, P]],
            compare_op=mybir.AluOpType.is_ge,
            fill=-1e9, base=0, channel_multiplier=-1)

        # ---- persistent storage --------------------------------------------
        qt_pool = ctx.enter_context(tc.tile_pool(name="qt", bufs=1))
        kt_pool = ctx.enter_context(tc.tile_pool(name="kt", bufs=1))
        v_pool = ctx.enter_context(tc.tile_pool(name="v", bufs=1))
        ct_pool = ctx.enter_context(tc.tile_pool(name="ct", bufs=1))
        xt_pool = ctx.enter_context(tc.tile_pool(name="xtp", bufs=1))
        w_pool = ctx.enter_context(tc.tile_pool(name="wp", bufs=1))
        wot_pool = ctx.enter_context(tc.tile_pool(name="wotp", bufs=1))
        qt_t = [qt_pool.tile([P, S], MMT, name=f"qt{j}", tag=f"qt{j}") for j in range(N_DL)]
        kt_t = [kt_pool.tile([P, S], MMT, name=f"kt{j}", tag=f"kt{j}") for j in range(N_DL)]
        v_t = [v_pool.tile([P, HH * VW], MMT, name=f"v{i}", tag=f"v{i}") for i in range(N_ST)]
        ct_t = [ct_pool.tile([P, S], MMT, name=f"ct{j}", tag=f"ct{j}") for j in range(N_DL)]

        # input DMAs (gpsimd SWDGE casts f32 -> bf16), roughly in first-use order
        xt_sb = []
        for i in range(N_DI):
            xti = xt_pool.tile([P, S], MMT, name=f"xt{i}", tag=f"xt{i}")
            _ldw(nc, xti[:], xt[i * P:(i + 1) * P, :])
            xt_sb.append(xti)
        w_sb = {}
        for nm, w_dram in (("q", wq), ("k", wk), ("v", wv)):
            for i in range(N_DI):
                wi = w_pool.tile([P, DL], MMT, name=f"w{nm}{i}", tag=f"w{nm}{i}")
                _ldw(nc, wi[:], w_dram[i * P:(i + 1) * P, :])
                w_sb[nm, i] = wi
        wot_sb = []
        for j in range(N_DL):
            wj = wot_pool.tile([P, O_], MMT, name=f"wot{j}", tag=f"wot{j}")
            _ldw(nc, wj[:], wot[j * P:(j + 1) * P, :])
            wot_sb.append(wj)

        pps = ctx.enter_context(tc.tile_pool(name="pps", bufs=2, space="PSUM"))
        exp_pool = ctx.enter_context(tc.tile_pool(
            name="exp", bufs=int(os.environ.get("MHA_EXBUFS", "12"))))
        z_pool = ctx.enter_context(tc.tile_pool(name="zp", bufs=4))
        sc_pool = ctx.enter_context(tc.tile_pool(name="scps", bufs=2, space="PSUM"))
        ctx_pool = ctx.enter_context(tc.tile_pool(name="ctxps", bufs=1, space="PSUM"))
        out_pool = ctx.enter_context(tc.tile_pool(name="outp", bufs=3))

        # ---- projection substep stream -------------------------------------
        # Substeps of ~2 matmuls each; attention emission drains/trickles them
        # so the PE never idles during the ACT-bound attention stretches.
        proj_steps = []
        marks = {}
        live = {}

        def make_qk_steps(nm, dst, dq):
            for qch in (0, 1):
                for di in range(N_DI):
                    def step(nm=nm, dst=dst, dq=dq, qch=qch, di=di):
                        key = (nm, dq, qch)
                        if di == 0:
                            live[key] = [
                                pps.tile([P, QC], F32, tag="pp",
                                         name=f"pp_{nm}{dq}_{qch}_{i}")
                                for i in (0, 1)]
                        ps2 = live[key]
                        for i, qc in enumerate((2 * qch, 2 * qch + 1)):
                            nc.tensor.matmul(
                                ps2[i][:],
                                w_sb[nm, di][:, dq * P:(dq + 1) * P],
                                xt_sb[di][:, qc * QC:(qc + 1) * QC],
                                start=(di == 0), stop=(di == N_DI - 1))
                    proj_steps.append(step)

                def copy_step(nm=nm, dst=dst, dq=dq, qch=qch):
                    ps2 = live.pop((nm, dq, qch))
                    for i, qc in enumerate((2 * qch, 2 * qch + 1)):
                        nc.vector.tensor_copy(
                            dst[dq][:, qc * QC:(qc + 1) * QC], ps2[i][:])
                proj_steps.append(copy_step)

        def make_v_steps(sg):
            for sth in (0, 1):
                sts = (4 * sg + 2 * sth, 4 * sg + 2 * sth + 1)
                for di in range(N_DI):
                    def step(sth=sth, di=di, sts=sts, sg=sg):
                        key = ("v", sg, sth)
                        if di == 0:
                            live[key] = [
                                pps.tile([P, DL], F32, tag="pp",
                                         name=f"ppv{sg}_{sth}_{i}")
                                for i in (0, 1)]
                        ps2 = live[key]
                        for i, st in enumerate(sts):
                            nc.tensor.matmul(
                                ps2[i][:],
                                xt_sb[di][:, st * P:(st + 1) * P],
                                w_sb["v", di][:],
                                start=(di == 0), stop=(di == N_DI - 1))
                    proj_steps.append(step)

                def copy_step(sth=sth, sts=sts, sg=sg):
                    ps2 = live.pop(("v", sg, sth))
                    for i, st in enumerate(sts):
                        vv = v_t[st].rearrange("p (h w) -> p h w", w=VW)
                        nc.vector.tensor_copy(
                            vv[:, :, 0:HD],
                            ps2[i].rearrange("p (h w) -> p h w", w=HD))
                        nc.vector.tensor_copy(
                            vv[:, :, HD:VW],
                            ones_f.rearrange("p (h o) -> p h o", o=1))
                proj_steps.append(copy_step)

        for blk in range(N_DL):
            make_qk_steps("q", qt_t, blk)
            make_qk_steps("k", kt_t, blk)
            marks["qk", blk] = len(proj_steps)
            make_v_steps(blk)
            marks["v", blk] = len(proj_steps)

        pi = [0]

        def drain_to(idx):
            while pi[0] < idx:
                proj_steps[pi[0]]()
                pi[0] += 1

        def trickle(n):
            for _ in range(n):
                if pi[0] < len(proj_steps):
                    proj_steps[pi[0]]()
                    pi[0] += 1

        # ---- attention unit stream -----------------------------------------
        # Staircase order over (pr, qc) so projection chunks are consumed
        # evenly; groups g = key-tile pairs within a unit.
        unit_order = sorted(
            ((pr, qc) for pr in range(N_DL) for qc in range(N_QC)),
            key=lambda u: (u[0] + u[1], u[1]))
        units = []
        for pr, qc in unit_order:
            ng = 2 * (qc + 1)
            for g in range(ng):
                units.append((pr, qc, g, ng))

        state = {}

        def emit_scores(u):
            pr, qc, g, ng = u
            if g == 0:
                drain_to(marks["qk", pr])
                drain_to(marks["v", qc])
            sc2 = [sc_pool.tile([P, 2 * QC], F32, tag="sc",
                                name=f"sc{hi}_{pr}_{qc}_{g}")
                   for hi in (0, 1)]
            offs = []
            for j in (0, 1):
                kt = 2 * g + j
                d = max(0, kt * P - qc * QC)   # masked q prefix width
                offs.append(d)
                for hi in (0, 1):
                    bp = 64 * hi
                    nc.tensor.matmul(
                        sc2[hi][:, j * QC + d:(j + 1) * QC],
                        kt_t[pr][bp:bp + HD, kt * P:(kt + 1) * P],
                        qt_t[pr][bp:bp + HD, qc * QC + d:(qc + 1) * QC],
                        start=True, stop=True,
                        tile_position=None if NOTILE else (bp, 0))
                if OLDMASK and not NOMASK:
                    dd = kt * P - qc * QC
                    if 0 <= dd < QC:
                        col = j * QC + dd
                        for hi in (0, 1):
                            nc.vector.tensor_tensor(
                                sc2[hi][:, col:col + P],
                                sc2[hi][:, col:col + P],
                                mask128[:], mybir.AluOpType.add)
            ex2 = [exp_pool.tile([P, 2 * QC], MMT, tag="ex",
                                 name=f"ex{hi}_{pr}_{qc}_{g}")
                   for hi in (0, 1)]
            for hi in (0, 1):
                if offs[0] == offs[1]:
                    nc.scalar.activation(ex2[hi][:, offs[0]:2 * QC],
                                         sc2[hi][:, offs[0]:2 * QC],
                                         Exp, scale=0.125)
                else:
                    for j in (0, 1):
                        d = offs[j]
                        nc.scalar.activation(
                            ex2[hi][:, j * QC + d:(j + 1) * QC],
                            sc2[hi][:, j * QC + d:(j + 1) * QC],
                            Exp, scale=0.125)
            if not NOMASK and not OLDMASK:
                for j in (0, 1):
                    kt = 2 * g + j
                    dd = kt * P - qc * QC
                    if 0 <= dd < QC:  # kt on the diagonal band of qc
                        col = j * QC + dd
                        for hi in (0, 1):
                            nc.vector.tensor_mask(
                                ex2[hi][:, col:col + P],
                                ex2[hi][:, col:col + P],
                                mlen_c[:], moffs[:], -1)
            state[(pr, qc, g)] = (ex2, offs)

        def emit_ctx(u):
            if NOCTX:
                state.pop(u[:3], None)
                return
            pr, qc, g, ng = u
            if g == 0:
                state[(pr, qc, "ctx")] = [
                    ctx_pool.tile([VW, QC], F32, tag=f"ctx{hi}",
                                  name=f"ctx{hi}_{pr}_{qc}")
                    for hi in (0, 1)]
            ctx2 = state[(pr, qc, "ctx")]
            ex2, offs = state.pop((pr, qc, g))
            nkt = 2 * ng
            for j in (0, 1):
                kt = 2 * g + j
                d = offs[j]
                for hi in (0, 1):
                    h = 2 * pr + hi
                    nc.tensor.matmul(
                        ctx2[hi][0:VW, d:QC],
                        v_t[kt][:, h * VW:(h + 1) * VW],
                        ex2[hi][:, j * QC + d:(j + 1) * QC],
                        start=(kt == 0), stop=(kt == nkt - 1))
            if g == ng - 1:
                ctx2 = state.pop((pr, qc, "ctx"))
                for hi in (0, 1):
                    bp = 64 * hi
                    rec = z_pool.tile([1, QC], F32, tag="rec")
                    nc.vector.reciprocal(rec[:], ctx2[hi][HD:VW, :])
                    rzb = z_pool.tile([HD, QC], F32, tag="rzb")
                    nc.gpsimd.partition_broadcast(rzb[:], rec[:])
                    nc.vector.tensor_tensor(
                        ct_t[pr][bp:bp + HD, qc * QC:(qc + 1) * QC],
                        ctx2[hi][0:HD, :], rzb[:], mult)

        for i, u in enumerate(units):
            emit_scores(u)
            trickle(TRICKLE)
            if i >= STAGGER:
                emit_ctx(units[i - STAGGER])
        for u in units[-STAGGER:]:
            emit_ctx(u)
        drain_to(len(proj_steps))

        # ---- transposed out-projection --------------------------------------
        for oc in range(O_ // P):
            ob = out_pool.tile([P, S], F32, tag="ob")
            for qch in (0, 1):
                ps2 = [pps.tile([P, QC], F32, tag="pp", name=f"op{oc}_{qch}_{i}")
                       for i in (0, 1)]
                for dl in range(N_DL):
                    for i, qc in enumerate((2 * qch, 2 * qch + 1)):
                        nc.tensor.matmul(
                            ps2[i][:],
                            wot_sb[dl][:, oc * P:(oc + 1) * P],
                            ct_t[dl][:, qc * QC:(qc + 1) * QC],
                            start=(dl == 0), stop=(dl == N_DL - 1))
                for i, qc in enumerate((2 * qch, 2 * qch + 1)):
                    if DVEBIAS:
                        nc.vector.tensor_scalar(
                            ob[:, qc * QC:(qc + 1) * QC], ps2[i][:],
                            bot_sb[:, oc:oc + 1], None, mybir.AluOpType.add)
                    else:
                        nc.scalar.activation(
                            ob[:, qc * QC:(qc + 1) * QC], ps2[i][:],
                            Ident, bias=bot_sb[:, oc:oc + 1])
            nc.sync.dma_start(out[oc * P:(oc + 1) * P, :], ob[:])


_NC_CACHE = None


def _get_nc():
    global _NC_CACHE
    if _NC_CACHE is None:
        _NC_CACHE = build_nc()
    return _NC_CACHE


def make_in_maps(x, Wq, Wk, Wv, Wo, bo):
    in_maps = []
    xts = [np.ascontiguousarray(x[b].T) for b in range(B)]
    bot = np.ascontiguousarray(bo.reshape(O_ // P, P).T).astype(np.float32)
    zeros_bot = np.zeros((P, O_ // P), np.float32)
    for c in range(N_CORES):
        b, half = c // 2, c % 2
        d0 = half * DL
        in_maps.append({
            "xt": xts[b],
            "wq": np.ascontiguousarray(Wq[:, d0:d0 + DL]),
            "wk": np.ascontiguousarray(Wk[:, d0:d0 + DL]),
            "wv": np.ascontiguousarray(Wv[:, d0:d0 + DL]),
            "wot": np.ascontiguousarray(Wo[:, d0:d0 + DL].T),
            "bot": bot if half == 0 else zeros_bot,
        })
    return in_maps


def kernel(x, Wq, Wk, Wv, Wo, bo):
    x = np.asarray(x, np.float32)
    Wq = np.asarray(Wq, np.float32)
    Wk = np.asarray(Wk, np.float32)
    Wv = np.asarray(Wv, np.float32)
    Wo = np.asarray(Wo, np.float32)
    bo = np.asarray(bo, np.float32)
    nc = _get_nc()
    in_maps = make_in_maps(x, Wq, Wk, Wv, Wo, bo)
    res = run_bass_kernel_spmd(nc, in_maps, core_ids=list(range(N_CORES)))
    out = np.empty((B, S, O_), np.float32)
    for b in range(B):
        out[b] = (res.results[2 * b]["out"] + res.results[2 * b + 1]["out"]).T
    return out



# revision 4
# speedup vs baseline: 1.0621x; 1.0621x over previous
"""Causal multi-head attention on 8 trn2 NeuronCores.

Problem: B=4, S=2048, D=1024, H=16 heads (HD=64), causal softmax attention
with out-projection + bias.

Sharding (tensor-parallel over heads, data-parallel over batch):
  core c -> batch b = c // 2, head half = c % 2 (8 of 16 heads, 512 dims).
  Every core runs the IDENTICAL program on different data:
    - xt   : x[b].T cast to bf16       [1024, 2048]
    - wq/wk/wv : W[:, half slice] bf16 [1024, 512]
    - wot  : Wo[:, half slice].T bf16  [512, 1024]
    - bot  : bias, transposed layout [128, 8] f32 (bot[p, c] = bo[c*128+p]);
             real on even cores, zeros on odd cores.
  Core output: partial TRANSPOSED out-projection [1024, 2048]; host sums the
  two partials per batch and transposes (row-parallel out_proj reduction).

Kernel (per core), flash-style with transposed scores, all matmul operands
bf16 (PSUM accumulation stays fp32; rel-err gate is 2e-2):
  QT = wq.T @ x.T   [512, 2048]; KT likewise; V = x @ wv [2048, 512]
  augmented with a ones column per head (the 65th row of the ctx matmul
  then yields the softmax denominator Z).
  scoresT[k, q] per head = KT_h-slice^T @ QT_h -> psum [128 keys, q].
  Head PAIRS are computed concurrently via PE row tiling (64x128 mode,
  tile_position (0,0)/(64,0)) since the contraction dim is HD=64.
  exp on ACT (scale = 1/8 folded in) -> bf16 ex tiles; causal masking is
  applied POST-exp only on the 128-wide diagonal blocks via gpsimd
  affine_select (fill 0 where q < k), keeping both the ACT critical path
  and the DVE queue clear of mask work.
  ctx^T accumulated over key tiles (lhsT = V|ones, M=65); the softmax
  denominator is inverted with the fast DVE approx reciprocal (~5x cheaper
  than nc.vector.reciprocal), broadcast on gpsimd, multiplied on DVE into
  ct (bf16).  The transposed out-projection outT[o, q] = wot.T-chunk @ ctxT
  is interleaved into the projection trickle stream per q-chunk as soon as
  all head-blocks of that chunk are normalized, so the PE stays busy during
  the ACT-bound attention stretches and the serial tail shrinks.
"""

import os
from contextlib import ExitStack

import numpy as np
import ml_dtypes

import concourse.mybir as mybir
import concourse.tile as tile
from concourse import bacc
from concourse.bass_utils import run_bass_kernel_spmd

B, S, D, H = 4, 2048, 1024, 16
HD = 64          # head dim
DL = 512         # local head dims per core (8 heads)
HH = 8           # local heads
P = 128
QC = 512         # q chunk (moving free dim)
N_QC = S // QC   # 4
N_DI = D // P    # 8
N_DL = DL // P   # 4
N_ST = S // P    # 16 seq tiles
VW = HD + 1      # 65: V columns + ones column
O_ = 1024        # output dims (full)

F32 = mybir.dt.float32
BF16 = mybir.dt.bfloat16
N_CORES = 8


def build_nc():
    nc = bacc.Bacc("TRN2", target_bir_lowering=False, debug=False,
                   num_devices=N_CORES, num_swdge_queues=4)
    xt = nc.dram_tensor("xt", [D, S], BF16, kind="ExternalInput").ap()
    wq = nc.dram_tensor("wq", [D, DL], BF16, kind="ExternalInput").ap()
    wk = nc.dram_tensor("wk", [D, DL], BF16, kind="ExternalInput").ap()
    wv = nc.dram_tensor("wv", [D, DL], BF16, kind="ExternalInput").ap()
    wot = nc.dram_tensor("wot", [DL, O_], BF16, kind="ExternalInput").ap()
    bot = nc.dram_tensor("bot", [P, O_ // P], F32, kind="ExternalInput").ap()
    out = nc.dram_tensor("out", [O_, S], F32, kind="ExternalOutput").ap()

    repeat = int(os.environ.get("MHA_REPEAT", "1"))
    hwloop = int(os.environ.get("MHA_HWLOOP", "0"))
    with tile.TileContext(nc) as tc:
        if hwloop > 1:
            with tc.For_i(0, hwloop, 1):
                _emit(nc, tc, xt, wq, wk, wv, wot, bot, out)
        else:
            for _ in range(repeat):
                _emit(nc, tc, xt, wq, wk, wv, wot, bot, out)
    nc.compile()
    return nc


def _emit(nc, tc, xt, wq, wk, wv, wot, bot, out):
    Exp = mybir.ActivationFunctionType.Exp
    mult = mybir.AluOpType.mult
    add = mybir.AluOpType.add

    NOMASK = bool(os.environ.get("MHA_NOMASK"))
    NOTILE = bool(os.environ.get("MHA_NOTILE"))
    STAGGER = int(os.environ.get("MHA_STAGGER", "4"))
    TRICKLE = int(os.environ.get("MHA_TRICKLE", "3"))

    with ExitStack() as ctx:
        # ---- small constants -----------------------------------------------
        consts = ctx.enter_context(tc.tile_pool(name="consts", bufs=1))
        ones_f = consts.tile([P, HH], F32, tag="ones_f")
        nc.gpsimd.memset(ones_f[:], 1.0)
        bot_sb = consts.tile([P, O_ // P], F32, tag="bot_sb")
        nc.sync.dma_start(bot_sb[:], bot[:])

        # ---- persistent storage --------------------------------------------
        qt_pool = ctx.enter_context(tc.tile_pool(name="qt", bufs=1))
        kt_pool = ctx.enter_context(tc.tile_pool(name="kt", bufs=1))
        v_pool = ctx.enter_context(tc.tile_pool(name="v", bufs=1))
        ct_pool = ctx.enter_context(tc.tile_pool(name="ct", bufs=1))
        xt_pool = ctx.enter_context(tc.tile_pool(name="xtp", bufs=1))
        w_pool = ctx.enter_context(tc.tile_pool(name="wp", bufs=1))
        wot_pool = ctx.enter_context(tc.tile_pool(name="wotp", bufs=1))
        qt_t = [qt_pool.tile([P, S], BF16, name=f"qt{j}", tag=f"qt{j}") for j in range(N_DL)]
        kt_t = [kt_pool.tile([P, S], BF16, name=f"kt{j}", tag=f"kt{j}") for j in range(N_DL)]
        v_t = [v_pool.tile([P, HH * VW], BF16, name=f"v{i}", tag=f"v{i}") for i in range(N_ST)]
        ct_t = [ct_pool.tile([P, S], BF16, name=f"ct{j}", tag=f"ct{j}") for j in range(N_DL)]

        # input DMAs in consumption order: the first attention unit needs all
        # of wq/wk/wv/xt, so interleave per-di on two HWDGE queues (weights on
        # the scalar queue, xt on sync); wot arrives last (out-proj).
        xt_sb = []
        w_sb = {}
        for i in range(N_DI):
            for nm, w_dram in (("q", wq), ("k", wk), ("v", wv)):
                wi = w_pool.tile([P, DL], BF16, name=f"w{nm}{i}", tag=f"w{nm}{i}")
                nc.scalar.dma_start(wi[:], w_dram[i * P:(i + 1) * P, :])
                w_sb[nm, i] = wi
            xti = xt_pool.tile([P, S], BF16, name=f"xt{i}", tag=f"xt{i}")
            nc.sync.dma_start(xti[:], xt[i * P:(i + 1) * P, :])
            xt_sb.append(xti)
        wot_sb = []
        for j in range(N_DL):
            wj = wot_pool.tile([P, O_], BF16, name=f"wot{j}", tag=f"wot{j}")
            nc.scalar.dma_start(wj[:], wot[j * P:(j + 1) * P, :])
            wot_sb.append(wj)

        pps = ctx.enter_context(tc.tile_pool(name="pps", bufs=2, space="PSUM"))
        exp_pool = ctx.enter_context(tc.tile_pool(
            name="exp", bufs=int(os.environ.get("MHA_EXBUFS", "12"))))
        z_pool = ctx.enter_context(tc.tile_pool(name="zp", bufs=4))
        sc_pool = ctx.enter_context(tc.tile_pool(name="scps", bufs=2, space="PSUM"))
        ctx_pool = ctx.enter_context(tc.tile_pool(name="ctxps", bufs=1, space="PSUM"))
        out_pool = ctx.enter_context(tc.tile_pool(name="outp", bufs=4))

        # ---- projection / out-projection substep stream --------------------
        # Substeps of ~2 matmuls each; attention emission drains/trickles them
        # so the PE never idles during the ACT-bound attention stretches.
        proj_steps = []
        marks = {}
        live = {}

        def make_qk_steps(nm, dst, dq):
            for qch in (0, 1):
                for di in range(N_DI):
                    def step(nm=nm, dst=dst, dq=dq, qch=qch, di=di):
                        key = (nm, dq, qch)
                        if di == 0:
                            live[key] = [
                                pps.tile([P, QC], F32, tag="pp",
                                         name=f"pp_{nm}{dq}_{qch}_{i}")
                                for i in (0, 1)]
                        ps2 = live[key]
                        for i, qc in enumerate((2 * qch, 2 * qch + 1)):
                            nc.tensor.matmul(
                                ps2[i][:],
                                w_sb[nm, di][:, dq * P:(dq + 1) * P],
                                xt_sb[di][:, qc * QC:(qc + 1) * QC],
                                start=(di == 0), stop=(di == N_DI - 1))
                    proj_steps.append(step)

                def copy_step(nm=nm, dst=dst, dq=dq, qch=qch):
                    ps2 = live.pop((nm, dq, qch))
                    for i, qc in enumerate((2 * qch, 2 * qch + 1)):
                        nc.vector.tensor_copy(
                            dst[dq][:, qc * QC:(qc + 1) * QC], ps2[i][:])
                proj_steps.append(copy_step)

        def make_v_steps(sg):
            for sth in (0, 1):
                sts = (4 * sg + 2 * sth, 4 * sg + 2 * sth + 1)
                for di in range(N_DI):
                    def step(sth=sth, di=di, sts=sts, sg=sg):
                        key = ("v", sg, sth)
                        if di == 0:
                            live[key] = [
                                pps.tile([P, DL], F32, tag="pp",
                                         name=f"ppv{sg}_{sth}_{i}")
                                for i in (0, 1)]
                        ps2 = live[key]
                        for i, st in enumerate(sts):
                            nc.tensor.matmul(
                                ps2[i][:],
                                xt_sb[di][:, st * P:(st + 1) * P],
                                w_sb["v", di][:],
                                start=(di == 0), stop=(di == N_DI - 1))
                    proj_steps.append(step)

                def copy_step(sth=sth, sts=sts, sg=sg):
                    ps2 = live.pop(("v", sg, sth))
                    for i, st in enumerate(sts):
                        vv = v_t[st].rearrange("p (h w) -> p h w", w=VW)
                        nc.vector.tensor_copy(
                            vv[:, :, 0:HD],
                            ps2[i].rearrange("p (h w) -> p h w", w=HD))
                        nc.vector.tensor_copy(
                            vv[:, :, HD:VW],
                            ones_f.rearrange("p (h o) -> p h o", o=1))
                proj_steps.append(copy_step)

        for blk in range(N_DL):
            make_qk_steps("q", qt_t, blk)
            make_qk_steps("k", kt_t, blk)
            marks["qk", blk] = len(proj_steps)
            make_v_steps(blk)
            marks["v", blk] = len(proj_steps)

        def make_outproj_steps(qc):
            # transposed out-projection for one q-chunk; appended to the step
            # stream once all N_DL head-blocks of chunk qc are normalized.
            for oc in range(O_ // P):
                for half in (0, 1):
                    def mm_step(oc=oc, half=half, qc=qc):
                        key = ("op", qc, oc)
                        if half == 0:
                            live[key] = pps.tile(
                                [P, QC], F32, tag="pp", name=f"op{qc}_{oc}")
                        ps = live[key]
                        for dl in (2 * half, 2 * half + 1):
                            nc.tensor.matmul(
                                ps[:],
                                wot_sb[dl][:, oc * P:(oc + 1) * P],
                                ct_t[dl][:, qc * QC:(qc + 1) * QC],
                                start=(dl == 0), stop=(dl == N_DL - 1))
                    proj_steps.append(mm_step)

                def evict_step(oc=oc, qc=qc):
                    ps = live.pop(("op", qc, oc))
                    ob = out_pool.tile([P, QC], F32, tag="ob",
                                       name=f"ob{qc}_{oc}")
                    nc.vector.tensor_scalar(
                        ob[:], ps[:], bot_sb[:, oc:oc + 1], None, add)
                    nc.sync.dma_start(
                        out[oc * P:(oc + 1) * P, qc * QC:(qc + 1) * QC], ob[:])
                proj_steps.append(evict_step)

        pi = [0]

        def drain_to(idx):
            while pi[0] < idx:
                proj_steps[pi[0]]()
                pi[0] += 1

        def trickle(n):
            for _ in range(n):
                if pi[0] < len(proj_steps):
                    proj_steps[pi[0]]()
                    pi[0] += 1

        # ---- attention unit stream -----------------------------------------
        # Staircase order over (pr, qc) so projection chunks are consumed
        # evenly; groups g = key-tile pairs within a unit.
        unit_order = sorted(
            ((pr, qc) for pr in range(N_DL) for qc in range(N_QC)),
            key=lambda u: (u[0] + u[1], u[1]))
        units = []
        for pr, qc in unit_order:
            ng = 2 * (qc + 1)
            for g in range(ng):
                units.append((pr, qc, g, ng))

        state = {}
        qc_done = [0] * N_QC

        def emit_scores(u):
            pr, qc, g, ng = u
            if g == 0:
                drain_to(marks["qk", pr])
                drain_to(marks["v", qc])
            sc2 = [sc_pool.tile([P, 2 * QC], F32, tag="sc",
                                name=f"sc{hi}_{pr}_{qc}_{g}")
                   for hi in (0, 1)]
            offs = []
            for j in (0, 1):
                kt = 2 * g + j
                d = max(0, kt * P - qc * QC)   # masked q prefix width
                offs.append(d)
                for hi in (0, 1):
                    bp = 64 * hi
                    nc.tensor.matmul(
                        sc2[hi][:, j * QC + d:(j + 1) * QC],
                        kt_t[pr][bp:bp + HD, kt * P:(kt + 1) * P],
                        qt_t[pr][bp:bp + HD, qc * QC + d:(qc + 1) * QC],
                        start=True, stop=True,
                        tile_position=None if NOTILE else (bp, 0))
            ex2 = [exp_pool.tile([P, 2 * QC], BF16, tag="ex",
                                 name=f"ex{hi}_{pr}_{qc}_{g}")
                   for hi in (0, 1)]
            for hi in (0, 1):
                if offs[0] == offs[1]:
                    nc.scalar.activation(ex2[hi][:, offs[0]:2 * QC],
                                         sc2[hi][:, offs[0]:2 * QC],
                                         Exp, scale=0.125)
                else:
                    for j in (0, 1):
                        d = offs[j]
                        nc.scalar.activation(
                            ex2[hi][:, j * QC + d:(j + 1) * QC],
                            sc2[hi][:, j * QC + d:(j + 1) * QC],
                            Exp, scale=0.125)
            if not NOMASK:
                # post-exp causal zeroing of the 128-wide diagonal block:
                # keep ex[k, q'] (q' local to the block) where q' - k >= 0.
                for j in (0, 1):
                    kt = 2 * g + j
                    dd = kt * P - qc * QC
                    if 0 <= dd < QC:  # kt on the diagonal band of qc
                        col = j * QC + dd
                        for hi in (0, 1):
                            nc.gpsimd.affine_select(
                                out=ex2[hi][:, col:col + P],
                                in_=ex2[hi][:, col:col + P],
                                pattern=[[1, P]],
                                compare_op=mybir.AluOpType.is_ge,
                                fill=0.0, base=0, channel_multiplier=-1)
            state[(pr, qc, g)] = (ex2, offs)

        def emit_ctx(u):
            pr, qc, g, ng = u
            if g == 0:
                state[(pr, qc, "ctx")] = [
                    ctx_pool.tile([VW, QC], F32, tag=f"ctx{hi}",
                                  name=f"ctx{hi}_{pr}_{qc}")
                    for hi in (0, 1)]
            ctx2 = state[(pr, qc, "ctx")]
            ex2, offs = state.pop((pr, qc, g))
            nkt = 2 * ng
            for j in (0, 1):
                kt = 2 * g + j
                d = offs[j]
                for hi in (0, 1):
                    h = 2 * pr + hi
                    nc.tensor.matmul(
                        ctx2[hi][0:VW, d:QC],
                        v_t[kt][:, h * VW:(h + 1) * VW],
                        ex2[hi][:, j * QC + d:(j + 1) * QC],
                        start=(kt == 0), stop=(kt == nkt - 1))
            if g == ng - 1:
                ctx2 = state.pop((pr, qc, "ctx"))
                if os.environ.get("MHA_RECIP", "batch") == "exact":
                    # v1-validated direct path: per-hi [1, 512] reciprocal
                    # straight from PSUM (slow: ~3.3us of DVE each).
                    for hi in (0, 1):
                        bp = 64 * hi
                        rec = z_pool.tile([1, QC], F32, tag="rec")
                        nc.vector.reciprocal(rec[:], ctx2[hi][HD:VW, :])
                        rzb = z_pool.tile([HD, QC], F32, tag="rzb")
                        nc.gpsimd.partition_broadcast(rzb[:], rec[:])
                        nc.vector.tensor_tensor(
                            ct_t[pr][bp:bp + HD, qc * QC:(qc + 1) * QC],
                            ctx2[hi][0:HD, :], rzb[:], mult)
                else:
                    # Batched denominator path: evict ctx+Z to SBUF (frees
                    # PSUM fast), DMA the two [1,512] Z rows into a [8,128]
                    # layout so ONE exact reciprocal covers both heads at
                    # 128 elems/lane, DMA back, broadcast, multiply.
                    cu2 = []
                    for hi in (0, 1):
                        cu = z_pool.tile([VW, QC], F32, tag=f"cu{hi}",
                                         bufs=2, name=f"cu{hi}_{pr}_{qc}")
                        nc.vector.tensor_copy(cu[:], ctx2[hi][:])
                        cu2.append(cu)
                    zb = z_pool.tile([8, P], F32, tag="zb")
                    for hi in (0, 1):
                        nc.sync.dma_start(
                            zb[4 * hi:4 * hi + 4, :], cu2[hi][HD:VW, :])
                    zr = z_pool.tile([8, P], F32, tag="zr")
                    nc.vector.reciprocal(zr[:], zb[:])
                    for hi in (0, 1):
                        bp = 64 * hi
                        rzs = z_pool.tile([1, QC], F32, tag="rzs")
                        nc.sync.dma_start(rzs[:], zr[4 * hi:4 * hi + 4, :])
                        rzb = z_pool.tile([HD, QC], F32, tag="rzb")
                        nc.gpsimd.partition_broadcast(rzb[:], rzs[:])
                        nc.vector.tensor_tensor(
                            ct_t[pr][bp:bp + HD, qc * QC:(qc + 1) * QC],
                            cu2[hi][0:HD, :], rzb[:], mult)
                qc_done[qc] += 1
                if qc_done[qc] == N_DL:
                    make_outproj_steps(qc)

        for i, u in enumerate(units):
            emit_scores(u)
            trickle(TRICKLE)
            if i >= STAGGER:
                emit_ctx(units[i - STAGGER])
        for u in units[-STAGGER:]:
            emit_ctx(u)
            trickle(TRICKLE)
        drain_to(len(proj_steps))


_NC_CACHE = None


def _get_nc():
    global _NC_CACHE
    if _NC_CACHE is None:
        _NC_CACHE = build_nc()
    return _NC_CACHE


def make_in_maps(x, Wq, Wk, Wv, Wo, bo):
    bf = ml_dtypes.bfloat16
    in_maps = []
    xts = [np.ascontiguousarray(x[b].T).astype(bf) for b in range(B)]
    bot = np.ascontiguousarray(bo.reshape(O_ // P, P).T).astype(np.float32)
    zeros_bot = np.zeros((P, O_ // P), np.float32)
    for c in range(N_CORES):
        b, half = c // 2, c % 2
        d0 = half * DL
        in_maps.append({
            "xt": xts[b],
            "wq": np.ascontiguousarray(Wq[:, d0:d0 + DL]).astype(bf),
            "wk": np.ascontiguousarray(Wk[:, d0:d0 + DL]).astype(bf),
            "wv": np.ascontiguousarray(Wv[:, d0:d0 + DL]).astype(bf),
            "wot": np.ascontiguousarray(Wo[:, d0:d0 + DL].T).astype(bf),
            "bot": bot if half == 0 else zeros_bot,
        })
    return in_maps


def kernel(x, Wq, Wk, Wv, Wo, bo):
    x = np.asarray(x, np.float32)
    Wq = np.asarray(Wq, np.float32)
    Wk = np.asarray(Wk, np.float32)
    Wv = np.asarray(Wv, np.float32)
    Wo = np.asarray(Wo, np.float32)
    bo = np.asarray(bo, np.float32)
    nc = _get_nc()
    in_maps = make_in_maps(x, Wq, Wk, Wv, Wo, bo)
    res = run_bass_kernel_spmd(nc, in_maps, core_ids=list(range(N_CORES)))
    out = np.empty((B, S, O_), np.float32)
    for b in range(B):
        out[b] = (res.results[2 * b]["out"] + res.results[2 * b + 1]["out"]).T
    return out
